# revision 1
# baseline (speedup 1.0000x reference)
"""Trainium2 Bass kernel for nn_CombinedPairwiseCacheLoss.

Computes, on 8 NeuronCores, the circle-style pairwise cache loss:
    emb_n = l2norm(embedding)                       # [N, D]
    cache = concat(emb_n, old_cache_features)[:M]   # [M, D]
    dist  = emb_n @ cache.T                         # [N, M]
    ... masked positive/negative logits, per-row logsumexp, softplus, mean.

Sharding: the cache (M=10000 rows) is split column-wise into 8 slabs of 1250
(padded to 1280).  Each core computes its local GEMM tile [1024 x 1280] plus
local masked sum-exp partials (fixed-offset logsumexp, so cross-core combine
is a plain sum done on the host during the gather step).

Device math per element (d = cosine similarity, m = label-match mask in {0,1}):
    sum_n partial:  exp(30*d^2       - 30*m      - 30  )   # == exp(l_n - 25.2)
    sum_p partial:  exp(30*(d-1)^2   - 30*(1-m)  - 44.8)   # == exp(l_p - 40.0)
The m=0/1 mask gives the wrong-side entries an extra e^-30 suppression factor,
which is far below the 1e-5-level accuracy of everything else (validated
against the reference in f64).  Host: lse_n = 25.2 + log(sum_n),
lse_p = 40 + log(sum_p) after subtracting the analytically-known diagonal and
zero-pad contributions, then mean(softplus(lse_p + lse_n)).

Distance matmuls run in float32r (full-rate PE, ~19-bit mantissa), which
lands the final loss within ~5e-7 relative of the f32 reference.
"""

import os
import sys

for _p in ("/opt/trn_rl_repo", "/root/.axon_site/_ro/trn_rl_repo"):
    if os.path.isdir(_p) and _p not in sys.path:
        sys.path.insert(0, _p)

import numpy as np

import concourse.bacc as bacc
import concourse.tile as tile
from concourse import mybir
from concourse.bass_utils import run_bass_kernel_spmd

F32 = mybir.dt.float32
F32R = mybir.dt.float32r
AF = mybir.ActivationFunctionType
ALU = mybir.AluOpType

NCORES = 8
N = 1024
D = 1024
M = 10000
SLAB = 1250          # cache rows per core
SLABP = 1280         # padded to a multiple of 128
NPAD = SLABP - SLAB  # 30 zero-padded cache rows per core
JCHUNKS = [(0, 512), (512, 512), (1024, 256)]  # bank-aligned psum regions
NB_I = 8             # 1024 rows / 128

USE_F32R = True

_NC_CACHE = {}


def _build_nc(use_f32r=USE_F32R):
    nc = bacc.Bacc(
        "TRN2", target_bir_lowering=False, debug=False, num_devices=NCORES
    )
    MDT = F32R if use_f32r else F32
    embT = nc.dram_tensor("embT", [D, N], MDT, kind="ExternalInput").ap()
    slabT = nc.dram_tensor("slabT", [D, SLABP], MDT, kind="ExternalInput").ap()
    labB = nc.dram_tensor("labB", [128, SLABP], F32, kind="ExternalInput").ap()
    tgtC = nc.dram_tensor("tgtC", [128, NB_I], F32, kind="ExternalInput").ap()
    pselC = nc.dram_tensor("pselC", [128, NB_I], F32, kind="ExternalInput").ap()
    ident = nc.dram_tensor("ident", [128, 128], F32, kind="ExternalInput").ap()
    onesI = nc.dram_tensor("onesI", [128, 128], MDT, kind="ExternalInput").ap()
    out = nc.dram_tensor("out", [2, 128, NB_I], F32, kind="ExternalOutput").ap()

    def f32view(ap):
        return ap.bitcast(F32) if use_f32r else ap

    with tile.TileContext(nc) as tc:
        with (
            tc.tile_pool(name="persist", bufs=1) as P,
            tc.tile_pool(name="emb", bufs=1) as PEmb,
            tc.tile_pool(name="slab", bufs=1) as PSlab,
            tc.tile_pool(name="sqn", bufs=2) as Psq,
            tc.tile_pool(name="work", bufs=2) as W,
            tc.tile_pool(name="psum_d", bufs=2, space="PSUM") as PP,
            tc.tile_pool(name="psum_s", bufs=2, space="PSUM") as PPs,
        ):
            # constants
            biasn = P.tile([128, 1], F32)
            nc.vector.memset(biasn[:], -30.0)
            biasp = P.tile([128, 1], F32)
            nc.vector.memset(biasp[:], -44.8)
            neg1 = P.tile([128, 1], F32)
            nc.vector.memset(neg1[:], -1.0)
            scratch1 = P.tile([128, 1], F32)
            # dummy activations: pull the Square/Exp/Sqrt LUT loads off the
            # critical path (each costs ~1.3us on first use)
            nc.scalar.activation(scratch1[:], biasn[:], AF.Square)
            nc.scalar.activation(scratch1[:], biasn[:], AF.Exp)
            nc.scalar.activation(scratch1[:], scratch1[:], AF.Sqrt)

            # inputs — two DMA queues: embT + labB on HWDGE/sync,
            # slab + small tensors on SWDGE/gpsimd.
            ones = P.tile([128, 128], MDT)
            nc.gpsimd.dma_start(ones[:], onesI[:])
            tgt_sb = P.tile([128, NB_I], F32)
            nc.gpsimd.dma_start(tgt_sb[:], tgtC[:])
            psel_sb = P.tile([128, NB_I], F32)
            nc.gpsimd.dma_start(psel_sb[:], pselC[:])
            id_sb = P.tile([128, 128], F32)
            nc.gpsimd.dma_start(id_sb[:], ident[:])

            embT_sb = []
            for dd in range(8):
                t = PEmb.tile([128, N], MDT, name=f"embT{dd}", tag=f"embT{dd}")
                nc.sync.dma_start(t[:], embT[dd * 128 : (dd + 1) * 128, :])
                embT_sb.append(t)
            labB_sb = P.tile([128, SLABP], F32)
            nc.sync.dma_start(labB_sb[:], labB[:])
            slab_sb = []
            for dd in range(8):
                t = PSlab.tile([128, SLABP], MDT, name=f"slab{dd}", tag=f"slab{dd}")
                nc.gpsimd.dma_start(t[:], slabT[dd * 128 : (dd + 1) * 128, :])
                slab_sb.append(t)

            # ---- embedding row norms:  norms2[i] = sum_dd embT[dd, i]^2
            ps_norm = [
                PPs.tile([1, 512], F32, name=f"psn{h}", tag="pss") for h in range(2)
            ]
            for dd in range(8):
                sq = Psq.tile([128, N], MDT, name="sq", tag="sqn")
                if dd % 2 == 0:
                    nc.vector.tensor_mul(
                        sq[:], f32view(embT_sb[dd][:]), f32view(embT_sb[dd][:])
                    )
                else:
                    nc.scalar.activation(sq[:], f32view(embT_sb[dd][:]), AF.Square)
                for h in range(2):
                    nc.tensor.matmul(
                        ps_norm[h][:],
                        ones[:, 0:1],
                        sq[:, h * 512 : (h + 1) * 512],
                        start=(dd == 0),
                        stop=(dd == 7),
                    )
            n2_free = P.tile([1, N], MDT)
            for h in range(2):
                nc.scalar.copy(n2_free[0:1, h * 512 : (h + 1) * 512], ps_norm[h][:])

            # transpose norms2 into per-partition column layout [128, 8]
            ps_nc = PPs.tile([128, NB_I], F32, name="psnc", tag="pss")
            for ib in range(NB_I):
                nc.tensor.matmul(
                    ps_nc[:, ib : ib + 1],
                    f32view(n2_free[0:1, ib * 128 : (ib + 1) * 128]),
                    f32view(ones[0:1, 0:1]),
                    start=True,
                    stop=True,
                )
            n2_col = P.tile([128, NB_I], F32)
            nc.scalar.copy(n2_col[:], ps_nc[:])
            inv2 = P.tile([128, NB_I], F32)
            nc.vector.reciprocal(inv2[:], n2_col[:])
            rinv = P.tile([128, NB_I], F32)
            nc.scalar.activation(rinv[:], inv2[:], AF.Sqrt)

            # scol = psel * (rinv - 1) + 1  (per-core column scale for the raw
            # embedding block inside core 0's cache slab; identity elsewhere)
            sc0 = P.tile([128, NB_I], F32)
            nc.vector.tensor_scalar(sc0[:], rinv[:], -1.0, None, ALU.add)
            sc1 = P.tile([128, NB_I], F32)
            nc.vector.tensor_mul(sc1[:], sc0[:], psel_sb[:])
            scol_c = P.tile([128, NB_I], F32)
            nc.vector.tensor_scalar(scol_c[:], sc1[:], 1.0, None, ALU.add)

            # transpose [128, 8] columns into a [1, 1024] free-layout row:
            # scol_c[:, b].T @ I gives row b*128..(b+1)*128
            scol_free = P.tile([1, N], MDT)
            for h in range(2):
                ps_f = PPs.tile([1, 512], F32, name=f"psf{h}", tag="pss")
                for bb in range(4):
                    b = h * 4 + bb
                    nc.tensor.matmul(
                        ps_f[0:1, bb * 128 : (bb + 1) * 128],
                        scol_c[:, b : b + 1],
                        id_sb[:],
                        start=True,
                        stop=True,
                    )
                nc.scalar.copy(scol_free[0:1, h * 512 : (h + 1) * 512], ps_f[:])

            # broadcast scol [1, 1024] -> [128, 1024]
            scolB = P.tile([128, N], F32)
            for h in range(2):
                ps_b = PPs.tile([128, 512], F32, name=f"psb{h}", tag="pss")
                nc.tensor.matmul(
                    ps_b[:],
                    ones[0:1, :],
                    scol_free[0:1, h * 512 : (h + 1) * 512],
                    start=True,
                    stop=True,
                )
                nc.scalar.copy(scolB[:, h * 512 : (h + 1) * 512], ps_b[:])

            # scale the raw-embedding block of the cache slab (cols 0..1023)
            for dd in range(8):
                nc.vector.tensor_mul(
                    slab_sb[dd][:, 0:N], f32view(slab_sb[dd][:, 0:N]), scolB[:]
                )

            # ---- main loop: one 3-bank psum tile [128, 1280] per row block,
            # whole-width epilogue (one instruction per stage).
            acc_n = P.tile([128, NB_I], F32)
            acc_p = P.tile([128, NB_I], F32)
            for ib in range(NB_I):
                rinv_ib = rinv[:, ib : ib + 1]
                tgt_ib = tgt_sb[:, ib : ib + 1]
                ps_d = PP.tile([128, SLABP], F32, name="psd", tag="psd")
                for j0, jw in JCHUNKS:
                    for dd in range(8):
                        nc.tensor.matmul(
                            ps_d[:, j0 : j0 + jw],
                            embT_sb[dd][:, ib * 128 : (ib + 1) * 128],
                            slab_sb[dd][:, j0 : j0 + jw],
                            start=(dd == 0),
                            stop=(dd == 7),
                        )
                # q = (rinv*g)^2 ;  s2 = (rinv*g - 1)^2
                q = W.tile([128, SLABP], F32, name="q", tag="q")
                nc.scalar.activation(
                    q[:], ps_d[:], AF.Square, bias=0.0, scale=rinv_ib
                )
                s2 = W.tile([128, SLABP], F32, name="s2", tag="s2")
                nc.scalar.activation(
                    s2[:], ps_d[:], AF.Square, bias=neg1[:, 0:1], scale=rinv_ib
                )
                # zn = (lab == tgt) - q ; zp = (lab != tgt) - s2
                zn = W.tile([128, SLABP], F32, name="zn", tag="zn")
                nc.vector.scalar_tensor_tensor(
                    zn[:], labB_sb[:], tgt_ib, q[:], ALU.is_equal, ALU.subtract
                )
                zp = W.tile([128, SLABP], F32, name="zp", tag="zp")
                nc.vector.scalar_tensor_tensor(
                    zp[:], labB_sb[:], tgt_ib, s2[:], ALU.not_equal, ALU.subtract
                )
                # en = exp(-30*zn - 30) ; ep = exp(-30*zp - 44.8)
                en = W.tile([128, SLABP], F32, name="en", tag="en")
                nc.scalar.activation(
                    en[:],
                    zn[:],
                    AF.Exp,
                    bias=biasn[:, 0:1],
                    scale=-30.0,
                    accum_out=acc_n[:, ib : ib + 1],
                )
                ep = W.tile([128, SLABP], F32, name="ep", tag="ep")
                nc.scalar.activation(
                    ep[:],
                    zp[:],
                    AF.Exp,
                    bias=biasp[:, 0:1],
                    scale=-30.0,
                    accum_out=acc_p[:, ib : ib + 1],
                )

            nc.sync.dma_start(out[0, :, :], acc_n[:])
            nc.sync.dma_start(out[1, :, :], acc_p[:])

    nc.compile()
    return nc


def _get_nc():
    key = USE_F32R
    if key not in _NC_CACHE:
        _NC_CACHE[key] = _build_nc(key)
    return _NC_CACHE[key]


def _prepare_in_maps(embedding, old_cache_features, targets, old_cache_labels):
    emb = np.ascontiguousarray(np.asarray(embedding, dtype=np.float32))
    oc = np.ascontiguousarray(np.asarray(old_cache_features, dtype=np.float32))
    tg = np.asarray(targets).astype(np.float64)
    ol = np.asarray(old_cache_labels).astype(np.float64)
    cache_labels = np.concatenate([tg, ol])[:M]

    embT = np.ascontiguousarray(emb.T)
    ident = np.eye(128, dtype=np.float32)
    ones_arr = np.ones((128, 128), dtype=np.float32)
    tgtC = np.ascontiguousarray(tg.reshape(NB_I, 128).T.astype(np.float32))

    in_maps = []
    for k in range(NCORES):
        j0 = SLAB * k
        if k == 0:
            rows = np.concatenate([emb, oc[0 : SLAB - N]], axis=0)
        else:
            rows = oc[j0 - N : j0 - N + SLAB]
        slabT = np.zeros((D, SLABP), np.float32)
        slabT[:, :SLAB] = rows.T
        labs = np.full(SLABP, -1.0, np.float64)
        labs[:SLAB] = cache_labels[j0 : j0 + SLAB]
        labB = np.ascontiguousarray(
            np.broadcast_to(labs.astype(np.float32), (128, SLABP))
        )
        pselC = np.full((128, NB_I), 1.0 if k == 0 else 0.0, np.float32)
        in_maps.append(
            dict(
                embT=embT,
                slabT=slabT,
                labB=labB,
                tgtC=tgtC,
                pselC=pselC,
                ident=ident,
                onesI=ones_arr,
            )
        )
    return in_maps


def _postprocess(results):
    sn = np.zeros(N, np.float64)
    sp = np.zeros(N, np.float64)
    for k in range(NCORES):
        o = np.asarray(results[k]["out"], np.float64)  # [2, 128, 8]
        sn += o[0].T.reshape(N)
        sp += o[1].T.reshape(N)
    # Analytic corrections (see module docstring):
    #  - the self-match (diagonal) term appears once per row on core 0:
    #    exp(-30) in sum_n (label matches, m=1) and exp(-44.8) in sum_p.
    #  - each of the 8*30 zero-padded cache rows contributes exp(-30) to
    #    sum_n (label -1 never matches, d=0) and exp(-44.8) to sum_p.
    sn -= (1 + NCORES * NPAD) * np.exp(-30.0)
    sp -= (1 + NCORES * NPAD) * np.exp(-44.8)
    lse_n = 25.2 + np.log(np.maximum(sn, 1e-300))
    lse_p = 40.0 + np.log(np.maximum(sp, 1e-300))
    loss = np.mean(np.logaddexp(0.0, lse_p + lse_n))
    return np.float32(loss)


def _run(in_maps, trace=False, **kwargs):
    nc = _get_nc()
    return run_bass_kernel_spmd(
        nc, in_maps, core_ids=list(range(NCORES)), trace=trace, **kwargs
    )


def kernel(embedding, old_cache_features, targets, old_cache_labels):
    in_maps = _prepare_in_maps(
        embedding, old_cache_features, targets, old_cache_labels
    )
    res = _run(in_maps)
    return _postprocess(res.results)



# revision 7
# speedup vs baseline: 1.3053x; 1.3053x over previous
"""Trainium2 Bass kernel for nn_CombinedPairwiseCacheLoss.

Computes, on 8 NeuronCores, the circle-style pairwise cache loss:
    emb_n = l2norm(embedding)                       # [N, D]
    cache = concat(emb_n, old_cache_features)[:M]   # [M, D]
    dist  = emb_n @ cache.T                         # [N, M]
    ... masked positive/negative logits, per-row logsumexp, softplus, mean.

Sharding: the cache (M=10000 rows) is split column-wise into 8 slabs of 1250
(padded to 1280).  Each core computes its local GEMM tile [1024 x 1280] plus
local masked sum-exp partials (fixed-offset logsumexp, so cross-core combine
is a plain sum done on the host during the gather step).

The embedding is l2-normalized on the host (cheap [N,D] numpy op), and both
GEMM operands ship as fp16 (full-rate PE; validated 4.7e-6 end-to-end loss
error).  With d the cosine similarity and m = (cache_label == row_target):
    en = exp(30*d^2 - 30*m - 30  )  == exp(logit_n - 25.2)
    ep = exp(30*d^2 - 60*d + 30*m - 44.8)  == exp(logit_p - 40.0)
Device epilogue per 128-row block, fp16 intermediates so the DVE runs its
2x perf mode (fp16 holds labels 0..999 and the compare exactly):
    vector: g  = copy(d)            (psum -> sbuf fp16; frees psum early)
            u  = g*g
            zp = u - 2*g
            xn = (lab == tgt) - u
            xp = (lab == tgt) + zp
    scalar: en = Exp(-30*xn - 30)   + row-accumulate
            ep = Exp( 30*xp - 44.8) + row-accumulate
Host: subtract the analytically-known diagonal/zero-pad contributions, then
lse_n = 25.2 + log(sum_n), lse_p = 40 + log(sum_p),
loss = mean(softplus(lse_p + lse_n)).

PSUM layout: main pool [128,1024] (2 banks) x3 bufs + tail pool [128,256]
(1 bank) x2 bufs = 8 banks.  Row-blocks 0-2 run contraction-step-outer so
the PE consumes (embT, slab) DMA bundles in arrival order (3 DMA queues:
sync/scalar HWDGE + gpsimd SWDGE, round-robin by contraction block).
"""

import os
import sys

for _p in ("/opt/trn_rl_repo", "/root/.axon_site/_ro/trn_rl_repo"):
    if os.path.isdir(_p) and _p not in sys.path:
        sys.path.insert(0, _p)

import numpy as np

import concourse.bacc as bacc
import concourse.tile as tile
from concourse import mybir
from concourse.bass_utils import run_bass_kernel_spmd

F32 = mybir.dt.float32
F16 = mybir.dt.float16
AF = mybir.ActivationFunctionType
ALU = mybir.AluOpType

NCORES = 8
N = 1024
D = 1024
M = 10000
SLAB = 1250          # cache rows per core
SLABP = 1280         # padded to a multiple of 128
NPAD = SLABP - SLAB  # zero-padded cache rows per core
NB_I = 8             # 1024 rows / 128
NACC = NB_I + 2      # last row-block accumulates per j-chunk (3 cells)
MAIN = 1024          # psum main tile width (2 banks)
TAIL = SLABP - MAIN  # psum tail tile width (1 bank)

_NC_CACHE = {}


def _build_nc():
    nc = bacc.Bacc(
        "TRN2", target_bir_lowering=False, debug=False, num_devices=NCORES
    )
    embT = nc.dram_tensor("embT", [D, N], F16, kind="ExternalInput").ap()
    slabT = nc.dram_tensor("slabT", [D, SLABP], F16, kind="ExternalInput").ap()
    labB = nc.dram_tensor("labB", [128, SLABP], F16, kind="ExternalInput").ap()
    tgtC = nc.dram_tensor("tgtC", [128, NB_I], F16, kind="ExternalInput").ap()
    out = nc.dram_tensor("out", [2, 128, NACC], F32, kind="ExternalOutput").ap()

    with tile.TileContext(nc) as tc:
        with (
            tc.tile_pool(name="persist", bufs=1) as P,
            tc.tile_pool(name="emb", bufs=1) as PEmb,
            tc.tile_pool(name="slab", bufs=1) as PSlab,
            tc.tile_pool(name="work", bufs=2) as W,
            tc.tile_pool(name="psum_m", bufs=3, space="PSUM") as PPm,
            tc.tile_pool(name="psum_t", bufs=2, space="PSUM") as PPt,
        ):
            # constants + Exp LUT warmup off the critical path (~1.3us)
            biasn = P.tile([128, 1], F32)
            nc.vector.memset(biasn[:], -30.0)
            biasp = P.tile([128, 1], F32)
            nc.vector.memset(biasp[:], -44.8)
            warm = P.tile([128, 1], F32)
            nc.scalar.activation(warm[:], biasn[:], AF.Exp)

            # ---- input DMAs: bundle dd -> (embT[dd], slab[dd]) round-robin
            # over the three DMA-capable queues, ascending dd so arrival
            # order matches the PE's contraction-step consumption order.
            tgt_sb = P.tile([128, NB_I], F16)
            nc.gpsimd.dma_start(tgt_sb[:], tgtC[:])
            embT_sb = []
            slab_sb = []
            for dd in range(8):
                t = PEmb.tile([128, N], F16, name=f"embT{dd}", tag=f"embT{dd}")
                embT_sb.append(t)
                s = PSlab.tile([128, SLABP], F16, name=f"slab{dd}", tag=f"slab{dd}")
                slab_sb.append(s)
            labB_sb = P.tile([128, SLABP], F16)
            qs = [nc.sync, nc.scalar, nc.gpsimd]
            for dd in range(8):
                q = qs[dd % 3]
                q.dma_start(embT_sb[dd][:], embT[dd * 128 : (dd + 1) * 128, :])
                q.dma_start(slab_sb[dd][:], slabT[dd * 128 : (dd + 1) * 128, :])
            nc.gpsimd.dma_start(labB_sb[:], labB[:])

            acc_n = P.tile([128, NACC], F32)
            acc_p = P.tile([128, NACC], F32)

            def mm_main(ib, psm, k):
                w = embT_sb[k][:, ib * 128 : (ib + 1) * 128]
                for j0 in (0, 512):
                    nc.tensor.matmul(
                        psm[:, j0 : j0 + 512],
                        w,
                        slab_sb[k][:, j0 : j0 + 512],
                        start=(k == 0),
                        stop=(k == 7),
                    )

            def mm_tail(ib, pst):
                for k in range(8):
                    nc.tensor.matmul(
                        pst[:],
                        embT_sb[k][:, ib * 128 : (ib + 1) * 128],
                        slab_sb[k][:, MAIN:SLABP],
                        start=(k == 0),
                        stop=(k == 7),
                    )

            def epilogue(ib, psm, pst, cells):
                # copy d out of psum first (2x-mode DVE copy; PSUM has a
                # single DVE read port, so d*d can't read psum twice) —
                # this also releases the psum banks early.
                g = W.tile([128, SLABP], F16, name="g", tag="g")
                nc.vector.tensor_copy(g[:, 0:MAIN], psm[:])
                nc.vector.tensor_copy(g[:, MAIN:SLABP], pst[:])
                u = W.tile([128, SLABP], F16, name="u", tag="u")
                zp = W.tile([128, SLABP], F16, name="zp", tag="zp")
                tgt_ib = tgt_sb[:, ib : ib + 1]
                xn = W.tile([128, SLABP], F16, name="xn", tag="xn")
                xp = W.tile([128, SLABP], F16, name="xp", tag="xp")
                en = W.tile([128, SLABP], F32, name="en", tag="en")
                ep = W.tile([128, SLABP], F32, name="ep", tag="ep")
                for c, j0, jw in cells:
                    gj = g[:, j0 : j0 + jw]
                    nc.vector.tensor_mul(u[:, j0 : j0 + jw], gj, gj)
                    nc.vector.scalar_tensor_tensor(
                        zp[:, j0 : j0 + jw], gj, -2.0, u[:, j0 : j0 + jw],
                        ALU.mult, ALU.add,
                    )
                    nc.vector.scalar_tensor_tensor(
                        xn[:, j0 : j0 + jw], labB_sb[:, j0 : j0 + jw], tgt_ib,
                        u[:, j0 : j0 + jw], ALU.is_equal, ALU.subtract,
                    )
                    nc.vector.scalar_tensor_tensor(
                        xp[:, j0 : j0 + jw], labB_sb[:, j0 : j0 + jw], tgt_ib,
                        zp[:, j0 : j0 + jw], ALU.is_equal, ALU.add,
                    )
                    nc.scalar.activation(
                        en[:, j0 : j0 + jw], xn[:, j0 : j0 + jw], AF.Exp,
                        bias=biasn[:, 0:1], scale=-30.0,
                        accum_out=acc_n[:, c : c + 1],
                    )
                    nc.scalar.activation(
                        ep[:, j0 : j0 + jw], xp[:, j0 : j0 + jw], AF.Exp,
                        bias=biasp[:, 0:1], scale=30.0,
                        accum_out=acc_p[:, c : c + 1],
                    )

            # group 0: row-blocks 0-2 interleaved, contraction-step outer,
            # so the PE tracks the DMA bundles as they land.
            mains = {}
            for ib in range(3):
                mains[ib] = PPm.tile([128, MAIN], F32, name=f"psm{ib}", tag="psm")
            for k in range(8):
                for ib in range(3):
                    mm_main(ib, mains[ib], k)
            for ib in range(3):
                pst = PPt.tile([128, TAIL], F32, name=f"pst{ib}", tag="pst")
                mm_tail(ib, pst)
                epilogue(ib, mains[ib], pst, [(ib, 0, SLABP)])

            # row-blocks 3-7: everything resident, one block at a time
            for ib in range(3, NB_I):
                psm = PPm.tile([128, MAIN], F32, name=f"psm{ib}", tag="psm")
                for k in range(8):
                    mm_main(ib, psm, k)
                pst = PPt.tile([128, TAIL], F32, name=f"pst{ib}", tag="pst")
                mm_tail(ib, pst)
                if ib < NB_I - 1:
                    epilogue(ib, psm, pst, [(ib, 0, SLABP)])
                else:
                    # chunked accumulation cells: short drain after last matmul
                    epilogue(ib, psm, pst,
                             [(7, 0, 512), (8, 512, 512), (9, MAIN, TAIL)])

            nc.sync.dma_start(out[0, :, :], acc_n[:])
            nc.sync.dma_start(out[1, :, :], acc_p[:])

    nc.compile()
    return nc


def _get_nc():
    if "v2" not in _NC_CACHE:
        _NC_CACHE["v2"] = _build_nc()
    return _NC_CACHE["v2"]


def _prepare_in_maps(embedding, old_cache_features, targets, old_cache_labels):
    emb = np.ascontiguousarray(np.asarray(embedding, dtype=np.float32))
    emb_n = emb / np.linalg.norm(emb, axis=1, keepdims=True)
    oc = np.asarray(old_cache_features, dtype=np.float32)
    tg = np.asarray(targets).astype(np.float64)
    ol = np.asarray(old_cache_labels).astype(np.float64)
    cache_labels = np.concatenate([tg, ol])[:M]

    embT = np.ascontiguousarray(emb_n.T.astype(np.float16))
    tgtC = np.ascontiguousarray(tg.reshape(NB_I, 128).T.astype(np.float16))
    cache = np.concatenate([emb_n, oc], axis=0)[:M]

    in_maps = []
    for k in range(NCORES):
        j0 = SLAB * k
        slabT = np.zeros((D, SLABP), np.float16)
        slabT[:, :SLAB] = cache[j0 : j0 + SLAB].T.astype(np.float16)
        labs = np.full(SLABP, -1.0, np.float64)
        labs[:SLAB] = cache_labels[j0 : j0 + SLAB]
        labB = np.ascontiguousarray(
            np.broadcast_to(labs.astype(np.float16), (128, SLABP))
        )
        in_maps.append(dict(embT=embT, slabT=slabT, labB=labB, tgtC=tgtC))
    return in_maps


def _postprocess(results):
    sn = np.zeros(N, np.float64)
    sp = np.zeros(N, np.float64)
    for k in range(NCORES):
        o = np.asarray(results[k]["out"], np.float64)  # [2, 128, NACC]
        on = np.concatenate([o[0][:, :7], o[0][:, 7:].sum(1, keepdims=True)], 1)
        op_ = np.concatenate([o[1][:, :7], o[1][:, 7:].sum(1, keepdims=True)], 1)
        sn += on.T.reshape(N)
        sp += op_.T.reshape(N)
    # Analytic corrections (see module docstring):
    #  - the self-match (diagonal) term appears once per row on core 0:
    #    exp(-30) in sum_n and exp(-44.8) in sum_p.
    #  - each of the 8*30 zero-padded cache rows contributes exp(-30) to
    #    sum_n (label -1 never matches, d=0) and exp(-44.8) to sum_p.
    sn -= (1 + NCORES * NPAD) * np.exp(-30.0)
    sp -= (1 + NCORES * NPAD) * np.exp(-44.8)
    lse_n = 25.2 + np.log(np.maximum(sn, 1e-300))
    lse_p = 40.0 + np.log(np.maximum(sp, 1e-300))
    loss = np.mean(np.logaddexp(0.0, lse_p + lse_n))
    return np.float32(loss)


def _run(in_maps, trace=False, **kwargs):
    nc = _get_nc()
    return run_bass_kernel_spmd(
        nc, in_maps, core_ids=list(range(NCORES)), trace=trace, **kwargs
    )


def kernel(embedding, old_cache_features, targets, old_cache_labels):
    in_maps = _prepare_in_maps(
        embedding, old_cache_features, targets, old_cache_labels
    )
    res = _run(in_maps)
    return _postprocess(res.results)


# revision 10
# speedup vs baseline: 1.3900x; 1.0649x over previous
"""Trainium2 Bass kernel for nn_CombinedPairwiseCacheLoss.

Computes, on 8 NeuronCores, the circle-style pairwise cache loss:
    emb_n = l2norm(embedding)                       # [N, D]
    cache = concat(emb_n, old_cache_features)[:M]   # [M, D]
    dist  = emb_n @ cache.T                         # [N, M]
    ... masked positive/negative logits, per-row logsumexp, softplus, mean.

Sharding: the cache (M=10000 rows) is split column-wise into 8 slabs of 1250
(padded to 1280).  Each core computes its local GEMM tile [1024 x 1280] plus
local masked sum-exp partials (fixed-offset logsumexp, so cross-core combine
is a plain sum done on the host during the gather step).

The embedding is l2-normalized on the host (cheap [N,D] numpy op), and both
GEMM operands ship as fp16 (full-rate PE; validated 4.7e-6 end-to-end loss
error).  With d the cosine similarity and m = (cache_label == row_target):
    en = exp(30*d^2 - 30*m - 30  )  == exp(logit_n - 25.2)
    ep = exp(30*d^2 - 60*d + 30*m - 44.8)  == exp(logit_p - 40.0)
Device epilogue per 128-row block, fp16 intermediates so the DVE runs its
2x perf mode (fp16 holds labels 0..999 and the compare exactly):
    vector: g  = copy(d)            (psum -> sbuf fp16; frees psum early)
            u  = g*g
            zp = u - 2*g
            xn = (lab == tgt) - u
            xp = (lab == tgt) + zp
    scalar: en = Exp(-30*xn - 30)   + row-accumulate
            ep = Exp( 30*xp - 44.8) + row-accumulate
Host: subtract the analytically-known diagonal/zero-pad contributions, then
lse_n = 25.2 + log(sum_n), lse_p = 40 + log(sum_p),
loss = mean(softplus(lse_p + lse_n)).

PSUM layout: main pool [128,1024] (2 banks) x3 bufs + tail pool [128,256]
(1 bank) x2 bufs = 8 banks.  Row-blocks 0-2 run contraction-step-outer so
the PE consumes (embT, slab) DMA bundles in arrival order (3 DMA queues:
sync/scalar HWDGE + gpsimd SWDGE, round-robin by contraction block).
"""

import os
import sys

for _p in ("/opt/trn_rl_repo", "/root/.axon_site/_ro/trn_rl_repo"):
    if os.path.isdir(_p) and _p not in sys.path:
        sys.path.insert(0, _p)

import numpy as np

import concourse.bacc as bacc
import concourse.tile as tile
from concourse import mybir
from concourse.bass_utils import run_bass_kernel_spmd

F32 = mybir.dt.float32
F16 = mybir.dt.float16
AF = mybir.ActivationFunctionType
ALU = mybir.AluOpType

NCORES = 8
N = 1024
D = 1024
M = 10000
SLAB = 1250          # cache rows per core
SLABP = 1280         # padded to a multiple of 128
NPAD = SLABP - SLAB  # zero-padded cache rows per core
NB_I = 8             # 1024 rows / 128
NACC = NB_I + 2      # last row-block accumulates per j-chunk (3 cells)
MAIN = 1024          # psum main tile width (2 banks)
TAIL = SLABP - MAIN  # psum tail tile width (1 bank)

_NC_CACHE = {}


def _build_nc():
    nc = bacc.Bacc(
        "TRN2", target_bir_lowering=False, debug=False, num_devices=NCORES
    )
    embT = nc.dram_tensor("embT", [D, N], F16, kind="ExternalInput").ap()
    slabT = nc.dram_tensor("slabT", [D, SLABP], F16, kind="ExternalInput").ap()
    labB = nc.dram_tensor("labB", [128, SLABP], F16, kind="ExternalInput").ap()
    tgtC = nc.dram_tensor("tgtC", [128, NB_I], F32, kind="ExternalInput").ap()
    out = nc.dram_tensor("out", [2, 128, NACC], F32, kind="ExternalOutput").ap()

    with tile.TileContext(nc) as tc:
        with (
            tc.tile_pool(name="persist", bufs=1) as P,
            tc.tile_pool(name="emb", bufs=1) as PEmb,
            tc.tile_pool(name="slab", bufs=1) as PSlab,
            tc.tile_pool(name="work", bufs=2) as W,
            tc.tile_pool(name="psum_m", bufs=3, space="PSUM") as PPm,
            tc.tile_pool(name="psum_t", bufs=2, space="PSUM") as PPt,
        ):
            # constants + Exp LUT warmup off the critical path (~1.3us)
            biasn = P.tile([128, 1], F32)
            nc.vector.memset(biasn[:], -30.0)
            biasp = P.tile([128, 1], F32)
            nc.vector.memset(biasp[:], -44.8)
            warm = P.tile([128, 1], F32)
            nc.scalar.activation(warm[:], biasn[:], AF.Square)
            nc.scalar.activation(warm[:], biasn[:], AF.Exp)

            # ---- input DMAs: bundle dd -> (embT[dd], slab[dd]) round-robin
            # over the three DMA-capable queues, ascending dd so arrival
            # order matches the PE's contraction-step consumption order.
            tgt_sb = P.tile([128, NB_I], F32)
            nc.gpsimd.dma_start(tgt_sb[:], tgtC[:])
            embT_sb = []
            slab_sb = []
            for dd in range(8):
                t = PEmb.tile([128, N], F16, name=f"embT{dd}", tag=f"embT{dd}")
                embT_sb.append(t)
                s = PSlab.tile([128, SLABP], F16, name=f"slab{dd}", tag=f"slab{dd}")
                slab_sb.append(s)
            labB_sb = P.tile([128, SLABP], F16)
            qs = [nc.sync, nc.scalar, nc.gpsimd]
            for dd in range(8):
                q = qs[dd % 3]
                q.dma_start(embT_sb[dd][:], embT[dd * 128 : (dd + 1) * 128, :])
                q.dma_start(slab_sb[dd][:], slabT[dd * 128 : (dd + 1) * 128, :])
            nc.gpsimd.dma_start(labB_sb[:], labB[:])

            acc_n = P.tile([128, NACC], F32)
            acc_p = P.tile([128, NACC], F32)

            def mm_main(ib, psm, k):
                w = embT_sb[k][:, ib * 128 : (ib + 1) * 128]
                for j0 in (0, 512):
                    nc.tensor.matmul(
                        psm[:, j0 : j0 + 512],
                        w,
                        slab_sb[k][:, j0 : j0 + 512],
                        start=(k == 0),
                        stop=(k == 7),
                    )

            def mm_tail(ib, pst):
                for k in range(8):
                    nc.tensor.matmul(
                        pst[:],
                        embT_sb[k][:, ib * 128 : (ib + 1) * 128],
                        slab_sb[k][:, MAIN:SLABP],
                        start=(k == 0),
                        stop=(k == 7),
                    )

            def epilogue(ib, psm, pst, cells):
                # scalar_tensor_tensor only runs the DVE at 1x; build the exp
                # args from ops with fast uop variants instead:
                #   scalar: u = Square(d)  (psum fast path, fp16 out)
                #   vector: h = -2d (ts, psum), m = (lab==tgt) (ts 4x),
                #           xn = m - u, zp = u + h, xp = m + zp (TT 2x fp16)
                u = W.tile([128, SLABP], F16, name="u", tag="u")
                h = W.tile([128, SLABP], F16, name="h", tag="h")
                m = W.tile([128, SLABP], F16, name="m", tag="m")
                zp = W.tile([128, SLABP], F16, name="zp", tag="zp")
                tgt_ib = tgt_sb[:, ib : ib + 1]
                xn = W.tile([128, SLABP], F16, name="xn", tag="xn")
                xp = W.tile([128, SLABP], F16, name="xp", tag="xp")
                en = W.tile([128, SLABP], F32, name="en", tag="en")
                ep = W.tile([128, SLABP], F32, name="ep", tag="ep")
                nc.scalar.activation(u[:, 0:MAIN], psm[:], AF.Square)
                nc.vector.tensor_scalar(h[:, 0:MAIN], psm[:], -2.0, None, ALU.mult)
                nc.scalar.activation(u[:, MAIN:SLABP], pst[:], AF.Square)
                nc.vector.tensor_scalar(h[:, MAIN:SLABP], pst[:], -2.0, None, ALU.mult)
                nc.vector.tensor_scalar(m[:], labB_sb[:], tgt_ib, None, ALU.is_equal)
                for c, j0, jw in cells:
                    sl = slice(j0, j0 + jw)
                    nc.vector.tensor_sub(xn[:, sl], m[:, sl], u[:, sl])
                    nc.vector.tensor_add(zp[:, sl], u[:, sl], h[:, sl])
                    nc.vector.tensor_add(xp[:, sl], m[:, sl], zp[:, sl])
                    nc.scalar.activation(
                        en[:, j0 : j0 + jw], xn[:, j0 : j0 + jw], AF.Exp,
                        bias=biasn[:, 0:1], scale=-30.0,
                        accum_out=acc_n[:, c : c + 1],
                    )
                    nc.scalar.activation(
                        ep[:, j0 : j0 + jw], xp[:, j0 : j0 + jw], AF.Exp,
                        bias=biasp[:, 0:1], scale=30.0,
                        accum_out=acc_p[:, c : c + 1],
                    )

            # group 0: row-blocks 0-2 interleaved, contraction-step outer,
            # so the PE tracks the DMA bundles as they land.
            mains = {}
            for ib in range(3):
                mains[ib] = PPm.tile([128, MAIN], F32, name=f"psm{ib}", tag="psm")
            for k in range(8):
                for ib in range(3):
                    mm_main(ib, mains[ib], k)
            for ib in range(3):
                pst = PPt.tile([128, TAIL], F32, name=f"pst{ib}", tag="pst")
                mm_tail(ib, pst)
                epilogue(ib, mains[ib], pst, [(ib, 0, SLABP)])

            # row-blocks 3-7: everything resident, one block at a time
            for ib in range(3, NB_I):
                psm = PPm.tile([128, MAIN], F32, name=f"psm{ib}", tag="psm")
                for k in range(8):
                    mm_main(ib, psm, k)
                pst = PPt.tile([128, TAIL], F32, name=f"pst{ib}", tag="pst")
                mm_tail(ib, pst)
                if ib < NB_I - 1:
                    epilogue(ib, psm, pst, [(ib, 0, SLABP)])
                else:
                    # chunked accumulation cells: short drain after last matmul
                    epilogue(ib, psm, pst,
                             [(7, 0, 512), (8, 512, 512), (9, MAIN, TAIL)])

            nc.sync.dma_start(out[0, :, :], acc_n[:])
            nc.sync.dma_start(out[1, :, :], acc_p[:])

    nc.compile()
    return nc


def _get_nc():
    if "v2" not in _NC_CACHE:
        _NC_CACHE["v2"] = _build_nc()
    return _NC_CACHE["v2"]


def _prepare_in_maps(embedding, old_cache_features, targets, old_cache_labels):
    emb = np.ascontiguousarray(np.asarray(embedding, dtype=np.float32))
    emb_n = emb / np.linalg.norm(emb, axis=1, keepdims=True)
    oc = np.asarray(old_cache_features, dtype=np.float32)
    tg = np.asarray(targets).astype(np.float64)
    ol = np.asarray(old_cache_labels).astype(np.float64)
    cache_labels = np.concatenate([tg, ol])[:M]

    embT = np.ascontiguousarray(emb_n.T.astype(np.float16))
    tgtC = np.ascontiguousarray(tg.reshape(NB_I, 128).T.astype(np.float32))
    cache = np.concatenate([emb_n, oc], axis=0)[:M]

    in_maps = []
    for k in range(NCORES):
        j0 = SLAB * k
        slabT = np.zeros((D, SLABP), np.float16)
        slabT[:, :SLAB] = cache[j0 : j0 + SLAB].T.astype(np.float16)
        labs = np.full(SLABP, -1.0, np.float64)
        labs[:SLAB] = cache_labels[j0 : j0 + SLAB]
        labB = np.ascontiguousarray(
            np.broadcast_to(labs.astype(np.float16), (128, SLABP))
        )
        in_maps.append(dict(embT=embT, slabT=slabT, labB=labB, tgtC=tgtC))
    return in_maps


def _postprocess(results):
    sn = np.zeros(N, np.float64)
    sp = np.zeros(N, np.float64)
    for k in range(NCORES):
        o = np.asarray(results[k]["out"], np.float64)  # [2, 128, NACC]
        on = np.concatenate([o[0][:, :7], o[0][:, 7:].sum(1, keepdims=True)], 1)
        op_ = np.concatenate([o[1][:, :7], o[1][:, 7:].sum(1, keepdims=True)], 1)
        sn += on.T.reshape(N)
        sp += op_.T.reshape(N)
    # Analytic corrections (see module docstring):
    #  - the self-match (diagonal) term appears once per row on core 0:
    #    exp(-30) in sum_n and exp(-44.8) in sum_p.
    #  - each of the 8*30 zero-padded cache rows contributes exp(-30) to
    #    sum_n (label -1 never matches, d=0) and exp(-44.8) to sum_p.
    sn -= (1 + NCORES * NPAD) * np.exp(-30.0)
    sp -= (1 + NCORES * NPAD) * np.exp(-44.8)
    lse_n = 25.2 + np.log(np.maximum(sn, 1e-300))
    lse_p = 40.0 + np.log(np.maximum(sp, 1e-300))
    loss = np.mean(np.logaddexp(0.0, lse_p + lse_n))
    return np.float32(loss)


def _run(in_maps, trace=False, **kwargs):
    nc = _get_nc()
    return run_bass_kernel_spmd(
        nc, in_maps, core_ids=list(range(NCORES)), trace=trace, **kwargs
    )


def kernel(embedding, old_cache_features, targets, old_cache_labels):
    in_maps = _prepare_in_maps(
        embedding, old_cache_features, targets, old_cache_labels
    )
    res = _run(in_maps)
    return _postprocess(res.results)


# revision 11
# speedup vs baseline: 1.7902x; 1.2879x over previous
"""Trainium2 Bass kernel for nn_CombinedPairwiseCacheLoss.

Computes, on 8 NeuronCores, the circle-style pairwise cache loss:
    emb_n = l2norm(embedding)                       # [N, D]
    cache = concat(emb_n, old_cache_features)[:M]   # [M, D]
    dist  = emb_n @ cache.T                         # [N, M]
    ... masked positive/negative logits, per-row logsumexp, softplus, mean.

Sharding: the cache (M=10000 rows) is split column-wise into 8 slabs of 1250
(padded to 1280).  Each core computes its local GEMM tile [1024 x 1280] plus
the local masked negative-side sum-exp partials (fixed-offset logsumexp, so
the cross-core combine is a plain sum done on the host during the gather).

Key split of work:
  - The positive-side logsumexp runs over label-MATCHED pairs only (~10 per
    row; targets/labels are host-known inputs), so the host computes those
    ~10k dot products directly in f64 — the positive side never touches the
    device.  (Unmatched entries carry NEG_INF logits in the reference and
    are dropped exactly.)
  - The negative side is dense: the device computes, per element,
        en = exp(30*d^2 - 30*m - 30) == exp(logit_n - 25.2)
    with m = (cache_label == row_target).  The mask term also suppresses the
    self-match diagonal (d=1 -> e^0 would swamp the f32 accumulator).
  - Host: lse_n = 25.2 + log(sum_n - analytic diag/pad terms),
    lse_p = 40 + log(sum_p), loss = mean(softplus(lse_p + lse_n)).

The embedding is l2-normalized on the host, and both GEMM operands ship as
fp16 (full-rate PE, half the DMA of f32; the host p-side replicates the same
fp16 input rounding).  Validated end-to-end: 5e-8 relative loss error.

Device epilogue per 128-row block (well under the 4.7us matmul pace):
    scalar: u  = Square(d)           (psum -> sbuf f32, also frees psum)
            en = Exp(-30*xn - 30) + row-accumulate
    vector: xn = (lab == tgt) - u    (scalar_tensor_tensor)

PSUM layout: main pool [128,1024] (2 banks) x3 bufs + tail pool [128,256]
(1 bank) x2 bufs = 8 banks.  Row-blocks 0-2 run contraction-step-outer so
the PE consumes (embT, slab) DMA bundles in arrival order (3 DMA queues:
sync/scalar HWDGE + gpsimd SWDGE, round-robin by contraction block).  The
last row-block accumulates per psum-chunk so the pipeline drain is short.
"""

import os
import sys

for _p in ("/opt/trn_rl_repo", "/root/.axon_site/_ro/trn_rl_repo"):
    if os.path.isdir(_p) and _p not in sys.path:
        sys.path.insert(0, _p)

import numpy as np

import concourse.bacc as bacc
import concourse.tile as tile
from concourse import mybir
from concourse.bass_utils import run_bass_kernel_spmd

F32 = mybir.dt.float32
F16 = mybir.dt.float16
AF = mybir.ActivationFunctionType
ALU = mybir.AluOpType

NCORES = 8
N = 1024
D = 1024
M = 10000
SLAB = 1250          # cache rows per core
SLABP = 1280         # padded to a multiple of 128
NPAD = SLABP - SLAB  # zero-padded cache rows per core
NB_I = 8             # 1024 rows / 128
NACC = NB_I + 2      # last row-block accumulates per j-chunk (3 cells)
MAIN = 1024          # psum main tile width (2 banks)
TAIL = SLABP - MAIN  # psum tail tile width (1 bank)

_NC_CACHE = {}
_HOST_SP = {"sp": None}  # host-computed positive-side sums, set by prepare


def _build_nc():
    nc = bacc.Bacc(
        "TRN2", target_bir_lowering=False, debug=False, num_devices=NCORES
    )
    embT = nc.dram_tensor("embT", [D, N], F16, kind="ExternalInput").ap()
    slabT = nc.dram_tensor("slabT", [D, SLABP], F16, kind="ExternalInput").ap()
    labB = nc.dram_tensor("labB", [128, SLABP], F32, kind="ExternalInput").ap()
    tgtC = nc.dram_tensor("tgtC", [128, NB_I], F32, kind="ExternalInput").ap()
    out = nc.dram_tensor("out", [128, NACC], F32, kind="ExternalOutput").ap()

    with tile.TileContext(nc) as tc:
        with (
            tc.tile_pool(name="persist", bufs=1) as P,
            tc.tile_pool(name="emb", bufs=1) as PEmb,
            tc.tile_pool(name="slab", bufs=1) as PSlab,
            tc.tile_pool(name="work", bufs=2) as W,
            tc.tile_pool(name="psum_m", bufs=3, space="PSUM") as PPm,
            tc.tile_pool(name="psum_t", bufs=2, space="PSUM") as PPt,
        ):
            # constants + ACT LUT warmups off the critical path (~1.3us each)
            biasn = P.tile([128, 1], F32)
            nc.vector.memset(biasn[:], -30.0)
            warm = P.tile([128, 1], F32)
            nc.scalar.activation(warm[:], biasn[:], AF.Square)
            nc.scalar.activation(warm[:], biasn[:], AF.Exp)

            # ---- input DMAs: bundle dd -> (embT[dd], slab[dd]) round-robin
            # over the three DMA-capable queues, ascending dd so arrival
            # order matches the PE's contraction-step consumption order.
            tgt_sb = P.tile([128, NB_I], F32)
            nc.gpsimd.dma_start(tgt_sb[:], tgtC[:])
            embT_sb = []
            slab_sb = []
            for dd in range(8):
                t = PEmb.tile([128, N], F16, name=f"embT{dd}", tag=f"embT{dd}")
                embT_sb.append(t)
                s = PSlab.tile([128, SLABP], F16, name=f"slab{dd}", tag=f"slab{dd}")
                slab_sb.append(s)
            labB_sb = P.tile([128, SLABP], F32)
            qs = [nc.sync, nc.scalar, nc.gpsimd]
            for dd in range(8):
                q = qs[dd % 3]
                q.dma_start(embT_sb[dd][:], embT[dd * 128 : (dd + 1) * 128, :])
                q.dma_start(slab_sb[dd][:], slabT[dd * 128 : (dd + 1) * 128, :])
            nc.gpsimd.dma_start(labB_sb[:], labB[:])

            acc_n = P.tile([128, NACC], F32)

            def mm_main(ib, psm, k):
                w = embT_sb[k][:, ib * 128 : (ib + 1) * 128]
                for j0 in (0, 512):
                    nc.tensor.matmul(
                        psm[:, j0 : j0 + 512],
                        w,
                        slab_sb[k][:, j0 : j0 + 512],
                        start=(k == 0),
                        stop=(k == 7),
                    )

            def mm_tail(ib, pst):
                for k in range(8):
                    nc.tensor.matmul(
                        pst[:],
                        embT_sb[k][:, ib * 128 : (ib + 1) * 128],
                        slab_sb[k][:, MAIN:SLABP],
                        start=(k == 0),
                        stop=(k == 7),
                    )

            def epilogue(ib, psm, pst, cells):
                u = W.tile([128, SLABP], F32, name="u", tag="u")
                xn = W.tile([128, SLABP], F32, name="xn", tag="xn")
                en = W.tile([128, SLABP], F32, name="en", tag="en")
                tgt_ib = tgt_sb[:, ib : ib + 1]
                nc.scalar.activation(u[:, 0:MAIN], psm[:], AF.Square)
                nc.scalar.activation(u[:, MAIN:SLABP], pst[:], AF.Square)
                for c, j0, jw in cells:
                    sl = slice(j0, j0 + jw)
                    nc.vector.scalar_tensor_tensor(
                        xn[:, sl], labB_sb[:, sl], tgt_ib, u[:, sl],
                        ALU.is_equal, ALU.subtract,
                    )
                    nc.scalar.activation(
                        en[:, sl], xn[:, sl], AF.Exp,
                        bias=biasn[:, 0:1], scale=-30.0,
                        accum_out=acc_n[:, c : c + 1],
                    )

            # group 0: row-blocks 0-2 interleaved, contraction-step outer,
            # so the PE tracks the DMA bundles as they land.
            mains = {}
            for ib in range(3):
                mains[ib] = PPm.tile([128, MAIN], F32, name=f"psm{ib}", tag="psm")
            for k in range(8):
                for ib in range(3):
                    mm_main(ib, mains[ib], k)
            for ib in range(3):
                pst = PPt.tile([128, TAIL], F32, name=f"pst{ib}", tag="pst")
                mm_tail(ib, pst)
                epilogue(ib, mains[ib], pst, [(ib, 0, SLABP)])

            # row-blocks 3-7: everything resident, one block at a time
            for ib in range(3, NB_I):
                psm = PPm.tile([128, MAIN], F32, name=f"psm{ib}", tag="psm")
                for k in range(8):
                    mm_main(ib, psm, k)
                pst = PPt.tile([128, TAIL], F32, name=f"pst{ib}", tag="pst")
                mm_tail(ib, pst)
                if ib < NB_I - 1:
                    epilogue(ib, psm, pst, [(ib, 0, SLABP)])
                else:
                    # chunked accumulation cells: short drain after last matmul
                    epilogue(ib, psm, pst,
                             [(7, 0, 512), (8, 512, 512), (9, MAIN, TAIL)])

            nc.sync.dma_start(out[:, :], acc_n[:])

    nc.compile()
    return nc


def _get_nc():
    if "v3" not in _NC_CACHE:
        _NC_CACHE["v3"] = _build_nc()
    return _NC_CACHE["v3"]


def _prepare_in_maps(embedding, old_cache_features, targets, old_cache_labels):
    emb = np.ascontiguousarray(np.asarray(embedding, dtype=np.float32))
    emb_n = emb / np.linalg.norm(emb, axis=1, keepdims=True)
    oc = np.asarray(old_cache_features, dtype=np.float32)
    tg = np.asarray(targets).astype(np.float64)
    ol = np.asarray(old_cache_labels).astype(np.float64)
    cache_labels = np.concatenate([tg, ol])[:M]
    cache = np.concatenate([emb_n, oc], axis=0)[:M]

    emb16 = emb_n.astype(np.float16)
    cache16 = cache.astype(np.float16)

    # ---- host positive side: label-matched pairs only (~10 per row), f64,
    # replicating the fp16 input rounding the device GEMM sees.
    pairs = np.argwhere(tg[:, None] == cache_labels[None, :])
    pairs = pairs[pairs[:, 0] != pairs[:, 1]]  # reference drops the diagonal
    dv = np.einsum(
        "ij,ij->i",
        emb16[pairs[:, 0]].astype(np.float64),
        cache16[pairs[:, 1]].astype(np.float64),
    )
    ep = np.exp(30.0 * (dv - 1.0) ** 2 - 44.8)
    sp = np.zeros(N, np.float64)
    np.add.at(sp, pairs[:, 0], ep)
    _HOST_SP["sp"] = sp

    embT = np.ascontiguousarray(emb16.T)
    tgtC = np.ascontiguousarray(tg.reshape(NB_I, 128).T.astype(np.float32))

    in_maps = []
    for k in range(NCORES):
        j0 = SLAB * k
        slabT = np.zeros((D, SLABP), np.float16)
        slabT[:, :SLAB] = cache16[j0 : j0 + SLAB].T
        labs = np.full(SLABP, -1.0, np.float64)
        labs[:SLAB] = cache_labels[j0 : j0 + SLAB]
        labB = np.ascontiguousarray(
            np.broadcast_to(labs.astype(np.float32), (128, SLABP))
        )
        in_maps.append(dict(embT=embT, slabT=slabT, labB=labB, tgtC=tgtC))
    return in_maps


def _postprocess(results):
    sn = np.zeros(N, np.float64)
    for k in range(NCORES):
        o = np.asarray(results[k]["out"], np.float64)  # [128, NACC]
        on = np.concatenate([o[:, :7], o[:, 7:].sum(1, keepdims=True)], 1)
        sn += on.T.reshape(N)
    # Analytic corrections:
    #  - the self-match (diagonal) term appears once per row on core 0:
    #    exp(30*d_ii^2 - 30 - 30) ~= exp(-30) since d_ii ~= 1.
    #  - each of the 8*30 zero-padded cache rows contributes exp(-30)
    #    (label -1 never matches, d = 0).
    sn -= (1 + NCORES * NPAD) * np.exp(-30.0)
    sp = _HOST_SP["sp"]
    lse_n = 25.2 + np.log(np.maximum(sn, 1e-300))
    lse_p = 40.0 + np.log(np.maximum(sp, 1e-300))
    loss = np.mean(np.logaddexp(0.0, lse_p + lse_n))
    return np.float32(loss)


def _run(in_maps, trace=False, **kwargs):
    nc = _get_nc()
    return run_bass_kernel_spmd(
        nc, in_maps, core_ids=list(range(NCORES)), trace=trace, **kwargs
    )


def kernel(embedding, old_cache_features, targets, old_cache_labels):
    in_maps = _prepare_in_maps(
        embedding, old_cache_features, targets, old_cache_labels
    )
    res = _run(in_maps)
    return _postprocess(res.results)


# revision 12
# speedup vs baseline: 1.8624x; 1.0403x over previous
"""Trainium2 Bass kernel for nn_CombinedPairwiseCacheLoss.

Computes, on 8 NeuronCores, the circle-style pairwise cache loss:
    emb_n = l2norm(embedding)                       # [N, D]
    cache = concat(emb_n, old_cache_features)[:M]   # [M, D]
    dist  = emb_n @ cache.T                         # [N, M]
    ... masked positive/negative logits, per-row logsumexp, softplus, mean.

Sharding: the cache (M=10000 rows) is split column-wise into 8 slabs of 1250
(padded to 1280 in dram, computed at 1250).  Each core computes its local
GEMM tile [1024 x 1250] plus the local masked negative-side sum-exp partials
(fixed-offset logsumexp, so the cross-core combine is a plain sum done on
the host during the gather).

Key split of work:
  - The positive-side logsumexp runs over label-MATCHED pairs only (~10 per
    row; targets/labels are host-known inputs), so the host computes those
    ~10k dot products directly in f64 — the positive side never touches the
    device.  (Unmatched entries carry NEG_INF logits in the reference and
    are dropped exactly.)
  - The negative side is dense: the device computes, per element,
        en = exp(30*d^2 - 30*m - 30) == exp(logit_n - 25.2)
    with m = (cache_label == row_target).  The mask term also suppresses the
    self-match diagonal (d=1 -> e^0 would swamp the f32 accumulator).
  - Host: lse_n = 25.2 + log(sum_n - analytic diag term),
    lse_p = 40 + log(sum_p), loss = mean(softplus(lse_p + lse_n)).

The embedding is l2-normalized on the host, and both GEMM operands ship as
fp16 (full-rate PE, half the DMA of f32; the host p-side replicates the same
fp16 input rounding).  Validated end-to-end: 5e-8 relative loss error.

Device epilogue per 128-row block (well under the 4.7us matmul pace):
    scalar: u  = Square(d)           (psum -> sbuf f32, also frees psum)
            en = Exp(-30*xn - 30) + row-accumulate
    vector: xn = (lab == tgt) - u    (scalar_tensor_tensor)

PSUM layout: main pool [128,1024] (2 banks) x3 bufs + tail pool [128,226]
(1 bank) x2 bufs = 8 banks.  Row-blocks 0-2 run contraction-step-outer so
the PE consumes (embT, slab) DMA bundles in arrival order (3 DMA queues:
sync/scalar HWDGE + gpsimd SWDGE, round-robin by contraction block); the
first-arriving embT halves carry just the weight columns those row-blocks
need.  The last row-block runs its tail chain first and accumulates per
psum-chunk so the pipeline drain is short.
"""

import os
import sys

for _p in ("/opt/trn_rl_repo", "/root/.axon_site/_ro/trn_rl_repo"):
    if os.path.isdir(_p) and _p not in sys.path:
        sys.path.insert(0, _p)

import numpy as np

import concourse.bacc as bacc
import concourse.tile as tile
from concourse import mybir
from concourse.bass_utils import run_bass_kernel_spmd

F32 = mybir.dt.float32
F16 = mybir.dt.float16
AF = mybir.ActivationFunctionType
ALU = mybir.AluOpType

NCORES = 8
N = 1024
D = 1024
M = 10000
SLAB = 1250          # cache rows per core (computed width)
SLABP = 1280         # dram padding to a multiple of 128
NB_I = 8             # 1024 rows / 128
NACC = NB_I + 2      # last row-block accumulates per j-chunk (3 cells)
MAIN = 1024          # psum main tile width (2 banks)
TAIL = SLAB - MAIN   # psum tail tile width (226 -> 1 bank)
WSPLIT = 384         # embT columns shipped in the first-phase bundles

_NC_CACHE = {}
_HOST_SP = {"sp": None}  # host-computed positive-side sums, set by prepare


def _build_nc():
    nc = bacc.Bacc(
        "TRN2", target_bir_lowering=False, debug=False, num_devices=NCORES
    )
    embT = nc.dram_tensor("embT", [D, N], F16, kind="ExternalInput").ap()
    slabT = nc.dram_tensor("slabT", [D, SLABP], F16, kind="ExternalInput").ap()
    labB = nc.dram_tensor("labB", [128, SLABP], F32, kind="ExternalInput").ap()
    tgtC = nc.dram_tensor("tgtC", [128, NB_I], F32, kind="ExternalInput").ap()
    out = nc.dram_tensor("out", [128, NACC], F32, kind="ExternalOutput").ap()

    with tile.TileContext(nc) as tc:
        with (
            tc.tile_pool(name="persist", bufs=1) as P,
            tc.tile_pool(name="emb", bufs=1) as PEmb,
            tc.tile_pool(name="slab", bufs=1) as PSlab,
            tc.tile_pool(name="work", bufs=2) as W,
            tc.tile_pool(name="psum_m", bufs=3, space="PSUM") as PPm,
            tc.tile_pool(name="psum_t", bufs=2, space="PSUM") as PPt,
        ):
            # ---- input DMAs first (nothing delays the transfers): bundle
            # dd -> (embT[dd][:, :WSPLIT], slab[dd]) round-robin over the
            # three DMA-capable queues, ascending dd so arrival order
            # matches the PE's contraction-step consumption order.  The
            # embT column tails (only needed by row-blocks 3+) and labB
            # (needed by the first epilogue) follow as a second phase.
            tgt_sb = P.tile([128, NB_I], F32)
            nc.gpsimd.dma_start(tgt_sb[:], tgtC[:])
            embT_sb = []
            slab_sb = []
            for dd in range(8):
                t = PEmb.tile([128, N], F16, name=f"embT{dd}", tag=f"embT{dd}")
                embT_sb.append(t)
                s = PSlab.tile([128, SLABP], F16, name=f"slab{dd}", tag=f"slab{dd}")
                slab_sb.append(s)
            labB_sb = P.tile([128, SLABP], F32)
            qs = [nc.sync, nc.scalar, nc.gpsimd]
            for dd in range(8):
                q = qs[dd % 3]
                r = slice(dd * 128, (dd + 1) * 128)
                q.dma_start(embT_sb[dd][:, 0:WSPLIT], embT[r, 0:WSPLIT])
                q.dma_start(slab_sb[dd][:], slabT[r, :])
            for dd in range(8):
                q = qs[dd % 3]
                r = slice(dd * 128, (dd + 1) * 128)
                q.dma_start(embT_sb[dd][:, WSPLIT:N], embT[r, WSPLIT:N])
            nc.gpsimd.dma_start(labB_sb[:], labB[:])

            # constants + ACT LUT warmups (after the DMA issues; they only
            # need to land before the first epilogue)
            biasn = P.tile([128, 1], F32)
            nc.vector.memset(biasn[:], -30.0)
            warm = P.tile([128, 1], F32)
            nc.scalar.activation(warm[:], biasn[:], AF.Square)
            nc.scalar.activation(warm[:], biasn[:], AF.Exp)

            acc_n = P.tile([128, NACC], F32)

            def mm_main(ib, psm, k):
                w = embT_sb[k][:, ib * 128 : (ib + 1) * 128]
                for j0 in (0, 512):
                    nc.tensor.matmul(
                        psm[:, j0 : j0 + 512],
                        w,
                        slab_sb[k][:, j0 : j0 + 512],
                        start=(k == 0),
                        stop=(k == 7),
                    )

            def mm_tail(ib, pst):
                for k in range(8):
                    nc.tensor.matmul(
                        pst[:],
                        embT_sb[k][:, ib * 128 : (ib + 1) * 128],
                        slab_sb[k][:, MAIN:SLAB],
                        start=(k == 0),
                        stop=(k == 7),
                    )

            def epilogue(ib, psm, pst, cells, per_cell_sq=False):
                u = W.tile([128, SLAB], F32, name="u", tag="u")
                xn = W.tile([128, SLAB], F32, name="xn", tag="xn")
                en = W.tile([128, SLAB], F32, name="en", tag="en")
                tgt_ib = tgt_sb[:, ib : ib + 1]
                if not per_cell_sq:
                    nc.scalar.activation(u[:, 0:MAIN], psm[:], AF.Square)
                    nc.scalar.activation(u[:, MAIN:SLAB], pst[:], AF.Square)
                for c, j0, jw in cells:
                    sl = slice(j0, j0 + jw)
                    if per_cell_sq:
                        src = pst[:] if j0 >= MAIN else psm[:, sl]
                        nc.scalar.activation(u[:, sl], src, AF.Square)
                    nc.vector.scalar_tensor_tensor(
                        xn[:, sl], labB_sb[:, sl], tgt_ib, u[:, sl],
                        ALU.is_equal, ALU.subtract,
                    )
                    nc.scalar.activation(
                        en[:, sl], xn[:, sl], AF.Exp,
                        bias=biasn[:, 0:1], scale=-30.0,
                        accum_out=acc_n[:, c : c + 1],
                    )

            # group 0: row-blocks 0-2 interleaved, contraction-step outer,
            # so the PE tracks the DMA bundles as they land.
            mains = {}
            for ib in range(3):
                mains[ib] = PPm.tile([128, MAIN], F32, name=f"psm{ib}", tag="psm")
            for k in range(8):
                for ib in range(3):
                    mm_main(ib, mains[ib], k)
            for ib in range(3):
                pst = PPt.tile([128, TAIL], F32, name=f"pst{ib}", tag="pst")
                mm_tail(ib, pst)
                epilogue(ib, mains[ib], pst, [(ib, 0, SLAB)])

            # row-blocks 3-6: everything resident, one block at a time
            for ib in range(3, NB_I - 1):
                psm = PPm.tile([128, MAIN], F32, name=f"psm{ib}", tag="psm")
                for k in range(8):
                    mm_main(ib, psm, k)
                pst = PPt.tile([128, TAIL], F32, name=f"pst{ib}", tag="pst")
                mm_tail(ib, pst)
                epilogue(ib, psm, pst, [(ib, 0, SLAB)])

            # last row-block: tail chain first, per-chunk squares and
            # accumulation cells -> short pipeline drain after the last matmul
            ib = NB_I - 1
            pst = PPt.tile([128, TAIL], F32, name=f"pst{ib}", tag="pst")
            mm_tail(ib, pst)
            psm = PPm.tile([128, MAIN], F32, name=f"psm{ib}", tag="psm")
            for k in range(8):
                mm_main(ib, psm, k)
            epilogue(ib, psm, pst,
                     [(9, MAIN, TAIL), (7, 0, 512), (8, 512, 512)],
                     per_cell_sq=True)

            nc.sync.dma_start(out[:, :], acc_n[:])

    nc.compile()
    return nc


def _get_nc():
    if "v3" not in _NC_CACHE:
        _NC_CACHE["v3"] = _build_nc()
    return _NC_CACHE["v3"]


def _prepare_in_maps(embedding, old_cache_features, targets, old_cache_labels):
    emb = np.ascontiguousarray(np.asarray(embedding, dtype=np.float32))
    emb_n = emb / np.linalg.norm(emb, axis=1, keepdims=True)
    oc = np.asarray(old_cache_features, dtype=np.float32)
    tg = np.asarray(targets).astype(np.float64)
    ol = np.asarray(old_cache_labels).astype(np.float64)
    cache_labels = np.concatenate([tg, ol])[:M]
    cache = np.concatenate([emb_n, oc], axis=0)[:M]

    emb16 = emb_n.astype(np.float16)
    cache16 = cache.astype(np.float16)

    # ---- host positive side: label-matched pairs only (~10 per row), f64,
    # replicating the fp16 input rounding the device GEMM sees.
    pairs = np.argwhere(tg[:, None] == cache_labels[None, :])
    pairs = pairs[pairs[:, 0] != pairs[:, 1]]  # reference drops the diagonal
    dv = np.einsum(
        "ij,ij->i",
        emb16[pairs[:, 0]].astype(np.float64),
        cache16[pairs[:, 1]].astype(np.float64),
    )
    ep = np.exp(30.0 * (dv - 1.0) ** 2 - 44.8)
    sp = np.zeros(N, np.float64)
    np.add.at(sp, pairs[:, 0], ep)
    _HOST_SP["sp"] = sp

    embT = np.ascontiguousarray(emb16.T)
    tgtC = np.ascontiguousarray(tg.reshape(NB_I, 128).T.astype(np.float32))

    in_maps = []
    for k in range(NCORES):
        j0 = SLAB * k
        slabT = np.zeros((D, SLABP), np.float16)
        slabT[:, :SLAB] = cache16[j0 : j0 + SLAB].T
        labs = np.full(SLABP, -1.0, np.float64)
        labs[:SLAB] = cache_labels[j0 : j0 + SLAB]
        labB = np.ascontiguousarray(
            np.broadcast_to(labs.astype(np.float32), (128, SLABP))
        )
        in_maps.append(dict(embT=embT, slabT=slabT, labB=labB, tgtC=tgtC))
    return in_maps


def _postprocess(results):
    sn = np.zeros(N, np.float64)
    for k in range(NCORES):
        o = np.asarray(results[k]["out"], np.float64)  # [128, NACC]
        on = np.concatenate([o[:, :7], o[:, 7:].sum(1, keepdims=True)], 1)
        sn += on.T.reshape(N)
    # Analytic correction: the self-match (diagonal) term appears once per
    # row on core 0: exp(30*d_ii^2 - 30 - 30) ~= exp(-30) since d_ii ~= 1.
    # (Zero-pad columns are never computed.)
    sn -= np.exp(-30.0)
    sp = _HOST_SP["sp"]
    lse_n = 25.2 + np.log(np.maximum(sn, 1e-300))
    lse_p = 40.0 + np.log(np.maximum(sp, 1e-300))
    loss = np.mean(np.logaddexp(0.0, lse_p + lse_n))
    return np.float32(loss)


def _run(in_maps, trace=False, **kwargs):
    nc = _get_nc()
    return run_bass_kernel_spmd(
        nc, in_maps, core_ids=list(range(NCORES)), trace=trace, **kwargs
    )


def kernel(embedding, old_cache_features, targets, old_cache_labels):
    in_maps = _prepare_in_maps(
        embedding, old_cache_features, targets, old_cache_labels
    )
    res = _run(in_maps)
    return _postprocess(res.results)
